# revision 23
# baseline (speedup 1.0000x reference)
"""Trainium2 Bass kernel for CrossAttentionFusion.

Math (kv seq_len == 1 collapses attention to two chained linear layers):
    eeg_att = ecg @ (Wo1 @ Wv1).T + (bv1 @ Wo1.T + bo1)
    eeg_out = LN(eeg + eeg_att) * g1 + beta1
    ecg_att = eeg @ (Wo2 @ Wv2).T + (bv2 @ Wo2.T + bo2)
    ecg_out = LN(ecg + ecg_att) * g2 + beta2
    out     = eeg_out @ WfL.T + ecg_out @ WfR.T + bf     (Wf = [WfL | WfR])

g/beta are folded into the fusion weights on the host:
    out = z1 @ (WfL*g1).T + z2 @ (WfR*g2).T + (bf + beta1@WfL.T + beta2@WfR.T)
where z = (a - mean(a)) * rsqrt(var(a) + eps) is the bare standardization.

The host pre-packs both modalities into one tensor per layout so each
128-row block needs exactly two load DMAs:
  xbn : [rows, 2*D] bf16 = [x1 | x2] natural rows, pre-scaled by SCALE
        (LN is scale-invariant; eps is scaled by SCALE^2 to stay exact)
  xbt : [blk*128 + p, m*D + c*128 + r] fp8 = x_m[blk*128 + r, c*128 + p]
        for m in (x2, x1) - transposed per-block chunks, matmul order
Attention weights are fp8-e4m3 scaled by SCALE (escapes e4m3 subnormals);
fusion weights stay bf16 (fp8 there fails the 2e-2 gate).

Per 128-row block:
  attention matmul in fp8 DoubleRow (x.T stationary [128,2,128], W.T
  moving [128,2,512], f32 PSUM) -> residual + LN stats on DVE ->
  z = a*rstd - mu*rstd on ScalarE into one [128, 2, D] tile -> single
  [128, 2048] transpose via the DMA xbar (SBUF->SBUF) -> bf16 fused
  matmul, emitted SKEW blocks late so the in-order PE stream never
  waits on the transpose chain -> f32 store.

Scheduling (the original strip kernel lost ~55us of PE time to DMA
cold-start, SP-sequencer congestion, and p-state re-ramps):
  - per-block loads in first-use order with a 4-block prefetch window,
    so the first attention matmul only waits on w1t + one 256KB chunk
  - loads + stores ride the SP HWDGE queue; the z-transpose rides the
    Activation HWDGE queue (dispatch costs ~600ns per DMA and in-order
    semaphore waits block everything behind them, so the transpose
    must not queue behind loads; its only wait is on z, produced by
    the same engine). Only the transpose goes there: load/store waits
    on the Activation queue couple the z critical path to tensor
    progress and lose ~100us.
  - a burst of dummy matmuls during the initial DMA wait ramps the PE
    clock (0.65 -> 2.4 GHz takes ~3us of continuous execution)
gpsimd (SWDGE) stays unused: its loads concurrent with DMA-transposes
wedge the device (unrecoverable).

Sharding: pure data parallel over the batch dim across 8 NeuronCores.
"""

import numpy as np
import ml_dtypes

import concourse.bass as bass
import concourse.mybir as mybir
import concourse.tile as tile
from concourse import bacc

B, D = 32768, 1024
N_CORES = 8
ROWS_PER_CORE = B // N_CORES
EPS = 1e-5
SCALE = 16.0  # fp8 scaling: w,x_nat carry x16; LN is scale-invariant (eps scaled too)
F32 = mybir.dt.float32
BF16 = mybir.dt.bfloat16
FP8 = mybir.dt.float8e4
BLK = 128  # row block (psum partition tile)
AF = mybir.ActivationFunctionType
ALU = mybir.AluOpType
DR = mybir.MatmulPerfMode.DoubleRow


def build_program(n_rows=ROWS_PER_CORE, use_b1=False, use_b2=False, use_bf=False):
    nc = bacc.Bacc("TRN2", target_bir_lowering=False, debug=False)
    n_blk = n_rows // BLK
    xbn = nc.dram_tensor("xbn", (n_rows, 2 * D), BF16, kind="ExternalInput").ap()
    xbt = nc.dram_tensor("xbt", (n_blk * 128, 2 * D), FP8, kind="ExternalInput").ap()
    w1t = nc.dram_tensor("w1t", (128, 8 * D), FP8, kind="ExternalInput").ap()
    w2t = nc.dram_tensor("w2t", (128, 8 * D), FP8, kind="ExternalInput").ap()
    wflt = nc.dram_tensor("wflt", (128, 8 * D), BF16, kind="ExternalInput").ap()
    wf8 = nc.dram_tensor("wf8", (128, 2 * D), FP8, kind="ExternalInput").ap()
    wfrt = nc.dram_tensor("wfrt", (128, 8 * D), BF16, kind="ExternalInput").ap()
    b1 = nc.dram_tensor("b1", (D,), F32, kind="ExternalInput").ap() if use_b1 else None
    b2 = nc.dram_tensor("b2", (D,), F32, kind="ExternalInput").ap() if use_b2 else None
    bfp = (
        nc.dram_tensor("bfp", (D,), F32, kind="ExternalInput").ap() if use_bf else None
    )
    out = nc.dram_tensor("out", (n_rows, D), F32, kind="ExternalOutput").ap()

    with tile.TileContext(nc) as tc:
        from contextlib import ExitStack

        with ExitStack() as ctx:
            consts = ctx.enter_context(tc.tile_pool(name="consts", bufs=1))
            xn_pool = ctx.enter_context(tc.tile_pool(name="xn", bufs=9))
            xt_pool = ctx.enter_context(tc.tile_pool(name="xt", bufs=9))
            work = ctx.enter_context(tc.tile_pool(name="work", bufs=6))
            zpool = ctx.enter_context(tc.tile_pool(name="z", bufs=6))
            ztpool = ctx.enter_context(tc.tile_pool(name="zt", bufs=10))
            opool = ctx.enter_context(tc.tile_pool(name="o", bufs=3))
            stats = ctx.enter_context(tc.tile_pool(name="stats", bufs=8))
            psum_mm = ctx.enter_context(
                tc.tile_pool(name="psum_mm", bufs=2, space="PSUM")
            )
            psum_o = ctx.enter_context(
                tc.tile_pool(name="psum_o", bufs=2, space="PSUM")
            )

            # --- PE clock warmup: dummy matmuls during the initial DMA wait.
            # The PE runs at 0.65/1.2 GHz until ~3us of continuous execution.
            warm = consts.tile([128, 256], BF16)
            nc.vector.memset(warm, 0.0)
            wps = psum_mm.tile([128, D], F32, name="ps_attn")
            for i in range(18):
                nc.tensor.matmul(
                    wps[:, 0:256], warm[:, 0:128], warm, start=True, stop=True
                )

            loads = {}  # (name, blk) -> tile

            def load_xt(j, eng=None):
                # transposed fp8 chunks [din_chunk=128, m, c, r=128]: m=0 is
                # x2 (attended by branch 0), m=1 is x1
                t = xt_pool.tile([128, 2, 8, BLK], FP8, name="xt")
                (eng or nc.sync).dma_start(
                    t,
                    xbt[j * 128 : (j + 1) * 128, :].rearrange(
                        "p (m c r) -> p m c r", m=2, c=8
                    ),
                )
                loads[("xt", j)] = t

            def load_xn(j, eng=None):
                # natural bf16 rows for the residual adds: m=0 is x1, m=1 is x2
                t = xn_pool.tile([128, 2, D], BF16, name="xn")
                (eng or nc.sync).dma_start(
                    t,
                    xbn[j * BLK : (j + 1) * BLK, :].rearrange("p (m n) -> p m n", m=2),
                )
                loads[("xn", j)] = t

            def load_block(j):
                load_xt(j)
                load_xn(j)

            # --- constants / weights. Each DMA dispatch serializes ~0.6-2.6us
            # on its sequencer and dispatch only starts at ~7us, so the cold
            # start is dispatch-bound: split the pre-staged loads across BOTH
            # HWDGE sequencers (they dispatch in parallel; none of these have
            # waits) in first-use order. The 4MB of fusion weights (first
            # needed by F(0) at ~30us) go last.
            def load_w(name_src, dt, eng, wname):
                t = consts.tile([128, 8, D], dt, name=wname)
                eng.dma_start(t, name_src.rearrange("p (c n) -> p c n", c=8))
                return t

            load_xt(0)
            w1t_sb = load_w(w1t, FP8, nc.sync, "w1t")
            load_xn(0)
            load_xt(1)
            w2t_sb = load_w(w2t, FP8, nc.sync, "w2t")
            load_xn(1)
            # fusion weights EARLY: F(0) must be runnable by ~27us so the
            # fused matmuls interleave with attention from the start -- an
            # attention-only burst outruns the z-chain (~6.3us/block) and
            # the resulting transpose backlog stalls the PE until ~130us
            wflt_sb = load_w(wflt, BF16, nc.sync, "wflt")
            wfrt_sb = load_w(wfrt, BF16, nc.sync, "wfrt")
            wf8_sb = consts.tile([128, 2, D], FP8, name="wf8")
            nc.sync.dma_start(wf8_sb, wf8.rearrange("p (c n) -> p c n", c=2))
            for _j in range(2, 7):
                load_block(_j)
            eps_sb = consts.tile([128, 1], F32)
            nc.vector.memset(eps_sb, EPS * SCALE * SCALE)
            b1_sb = b2_sb = bf_sb = None
            if use_b1:
                b1_sb = consts.tile([128, D], F32)
                nc.sync.dma_start(b1_sb, b1.partition_broadcast(128))
            if use_b2:
                b2_sb = consts.tile([128, D], F32)
                nc.sync.dma_start(b2_sb, b2.partition_broadcast(128))
            if use_bf:
                bf_sb = consts.tile([128, D], F32)
                nc.sync.dma_start(bf_sb, bfp.partition_broadcast(128))

            PF = 7  # blocks of load prefetch ahead of compute
            SKEW = 5  # blocks the fused matmul lags behind attention
            pending = []

            def emit_fused(r, zt, zt8):
                # out = z1 @ WfL'.T + z2 @ WfR'.T (+ bf')
                # z1 k-chunks 0,1 run as ONE fp8 DoubleRow matmul per half
                # (operands at natural scale so the f32 PSUM accumulation
                # matches the bf16 chunks; rel err 0.0105 -> 0.0152, gate 2e-2)
                po = psum_o.tile([128, D], F32, name="po")
                for br in range(2):
                    wt = wflt_sb if br == 0 else wfrt_sb
                    for c in range(2 if br == 0 else 0, 8):
                        lhsT = zt[:, 8 * br + c, :]
                        nc.tensor.matmul(
                            po[:, 0:512],
                            lhsT,
                            wt[:, c, 0:512],
                            start=(br == 0 and c == 2),
                            stop=False,
                        )
                        nc.tensor.matmul(
                            po[:, 512:1024],
                            lhsT,
                            wt[:, c, 512:1024],
                            start=(br == 0 and c == 2),
                            stop=False,
                        )
                nc.tensor.matmul(
                    po[:, 0:512], zt8, wf8_sb[:, :, 0:512],
                    start=False, stop=True, perf_mode=DR,
                )
                nc.tensor.matmul(
                    po[:, 512:1024], zt8, wf8_sb[:, :, 512:1024],
                    start=False, stop=True, perf_mode=DR,
                )
                o = opool.tile([128, D], F32, name="o")
                if bf_sb is not None:
                    nc.vector.tensor_add(o, po, bf_sb)
                else:
                    # on DVE, not ScalarE: a copy on the scalar engine makes
                    # the next block's z wait on fusion matmul completion
                    # (in-order engine stream), closing a feedback cycle with
                    # period > the 10.85us block time
                    nc.vector.tensor_copy(o, po)
                nc.sync.dma_start(out[r : r + BLK, :], o)

            for j in range(n_blk):
                if len(pending) > SKEW:
                    emit_fused(*pending.pop(0))
                if j + PF < n_blk:
                    load_block(j + PF)
                r = j * BLK
                z = zpool.tile([128, 2, D], BF16, name="z")
                for br in range(2):
                    xt_op = loads[("xt", j)][:, br]  # attended modality
                    res = loads[("xn", j)][:, br]
                    wt = w1t_sb if br == 0 else w2t_sb
                    bias_sb = b1_sb if br == 0 else b2_sb
                    # attended = x_other @ W.T    [128 rows, 1024]
                    # fp8 DoubleRow: 2 k-chunks per matmul (virtual K=256)
                    ps = psum_mm.tile([128, D], F32, name="ps_attn")
                    for c in range(4):
                        lhsT = xt_op[:, 2 * c : 2 * c + 2, :]
                        nc.tensor.matmul(
                            ps[:, 0:512],
                            lhsT,
                            wt[:, 2 * c : 2 * c + 2, 0:512],
                            start=(c == 0),
                            stop=(c == 3),
                            perf_mode=DR,
                        )
                        nc.tensor.matmul(
                            ps[:, 512:1024],
                            lhsT,
                            wt[:, 2 * c : 2 * c + 2, 512:1024],
                            start=(c == 0),
                            stop=(c == 3),
                            perf_mode=DR,
                        )
                    # a = residual + attended (+ bias), bf16 for 2x DVE reads
                    a = work.tile([128, D], BF16, name="a")
                    nc.vector.tensor_add(a, ps, res)
                    if bias_sb is not None:
                        nc.vector.tensor_add(a, a, bias_sb)
                    # layernorm statistics
                    st = stats.tile([128, 2, 6], F32, name="st")
                    nc.vector.bn_stats(st[:, 0, :], a[:, 0:512])
                    nc.vector.bn_stats(st[:, 1, :], a[:, 512:1024])
                    mv = stats.tile([128, 2], F32, name="mv")
                    nc.vector.bn_aggr(mv, st)
                    rstd = stats.tile([128, 1], F32, name="rstd")
                    nc.scalar.activation(rstd, mv[:, 1:2], AF.Sqrt, bias=eps_sb)
                    nc.vector.reciprocal(rstd, rstd)
                    # z = a*rstd - mean*rstd on the scalar engine
                    nmr = stats.tile([128, 1], F32, name="nmr")
                    nc.vector.tensor_scalar(
                        nmr, mv[:, 0:1], rstd, -1.0, op0=ALU.mult, op1=ALU.mult
                    )
                    nc.vector.tensor_scalar(
                        z[:, br], a, rstd, nmr, op0=ALU.mult, op1=ALU.add
                    )
                # transpose both halves of z in one DMA xbar pass
                # (SBUF->SBUF, 2-byte): zt[p, 8*br + c, r] = z[r, br, c*128 + p]
                zt = ztpool.tile([128, 16, BLK], BF16, name="zt")
                nc.scalar.dma_start(zt, z.rearrange("p m n -> p (m n)"), transpose=True)
                zt8 = ztpool.tile([128, 2, BLK], FP8, name="zt8")
                nc.scalar.copy(zt8, zt[:, 0:2, :])
                pending.append((r, zt, zt8))
            for args in pending:
                emit_fused(*args)
    nc.compile()
    return nc


def _host_prep(Wv1, bv1, Wo1, bo1, Wv2, bv2, Wo2, bo2, g1, beta1, g2, beta2, Wf, bf):
    f32 = np.float32
    bfd = ml_dtypes.bfloat16
    Wv1, Wo1, Wv2, Wo2, Wf = (np.asarray(a, f32) for a in (Wv1, Wo1, Wv2, Wo2, Wf))
    bv1, bo1, bv2, bo2, bf = (np.asarray(a, f32) for a in (bv1, bo1, bv2, bo2, bf))
    g1, beta1, g2, beta2 = (np.asarray(a, f32) for a in (g1, beta1, g2, beta2))

    W1 = Wo1 @ Wv1  # [dout, din]
    W2 = Wo2 @ Wv2
    b1 = bv1 @ Wo1.T + bo1
    b2 = bv2 @ Wo2.T + bo2
    WfL = Wf[:, :D] * g1[None, :]
    WfR = Wf[:, D:] * g2[None, :]
    bfp = bf + beta1 @ Wf[:, :D].T + beta2 @ Wf[:, D:].T

    f8 = ml_dtypes.float8_e4m3

    def wlay(wT, dt):
        # SBUF layout [p, c, n] flattened to [128, 8*D]: row p holds chunk
        # rows p, 128+p, ... so the DMA is one contiguous 8KB line per row
        return np.ascontiguousarray(
            wT.reshape(8, 128, D).transpose(1, 0, 2).reshape(128, 8 * D)
        ).astype(dt)

    weights = {
        "w1t": wlay(SCALE * W1.T, f8),
        "w2t": wlay(SCALE * W2.T, f8),
        "wflt": wlay(WfL.T, bfd),
        "wfrt": wlay(WfR.T, bfd),
        # z1 k-chunks 0,1 of the fused matmul in fp8 at natural scale
        "wf8": np.ascontiguousarray(
            WfL.T[:256].reshape(2, 128, D).transpose(1, 0, 2).reshape(128, 2 * D)
        ).astype(f8),
    }
    use_b1 = bool(np.any(b1 != 0))
    use_b2 = bool(np.any(b2 != 0))
    use_bf = bool(np.any(bfp != 0))
    if use_b1:
        weights["b1"] = SCALE * b1
    if use_b2:
        weights["b2"] = SCALE * b2
    if use_bf:
        weights["bfp"] = bfp
    return weights, use_b1, use_b2, use_bf


def _xt_blocks(x_core):
    """Per-block transposed fp8 chunks: [blk*128 + p, c*128 + r] =
    x[blk*128 + r, c*128 + p], flattened to [n_blk*128, D]."""
    n_blk = x_core.shape[0] // BLK
    return np.ascontiguousarray(
        x_core.astype(ml_dtypes.float8_e4m3)
        .reshape(n_blk, BLK, 8, 128)
        .transpose(0, 3, 2, 1)
    ).reshape(n_blk * 128, D)


def kernel(
    eeg_emb,
    ecg_emb,
    Wv1,
    bv1,
    Wo1,
    bo1,
    Wv2,
    bv2,
    Wo2,
    bo2,
    g1,
    beta1,
    g2,
    beta2,
    Wf,
    bf,
    _run_kwargs=None,
):
    from concourse.bass_utils import run_bass_kernel_spmd

    eeg = np.asarray(eeg_emb, np.float32)
    ecg = np.asarray(ecg_emb, np.float32)
    weights, use_b1, use_b2, use_bf = _host_prep(
        Wv1, bv1, Wo1, bo1, Wv2, bv2, Wo2, bo2, g1, beta1, g2, beta2, Wf, bf
    )
    nc = build_program(ROWS_PER_CORE, use_b1, use_b2, use_bf)
    bfd = ml_dtypes.bfloat16
    in_maps = []
    for i in range(N_CORES):
        sl = slice(i * ROWS_PER_CORE, (i + 1) * ROWS_PER_CORE)
        e, c = eeg[sl], ecg[sl]
        xbn = np.concatenate(
            [(SCALE * e).astype(bfd), (SCALE * c).astype(bfd)], axis=1
        )
        xbt = np.concatenate([_xt_blocks(c), _xt_blocks(e)], axis=1)
        in_maps.append({"xbn": xbn, "xbt": xbt, **weights})
    res = run_bass_kernel_spmd(
        nc, in_maps, core_ids=list(range(N_CORES)), **(_run_kwargs or {})
    )
    out = np.concatenate([r["out"] for r in res.results], axis=0)
    if _run_kwargs:
        kernel.last_results = res
    return out


# revision 24
# speedup vs baseline: 1.0623x; 1.0623x over previous
"""Trainium2 Bass kernel for CrossAttentionFusion.

Math (kv seq_len == 1 collapses attention to two chained linear layers):
    eeg_att = ecg @ (Wo1 @ Wv1).T + (bv1 @ Wo1.T + bo1)
    eeg_out = LN(eeg + eeg_att) * g1 + beta1
    ecg_att = eeg @ (Wo2 @ Wv2).T + (bv2 @ Wo2.T + bo2)
    ecg_out = LN(ecg + ecg_att) * g2 + beta2
    out     = eeg_out @ WfL.T + ecg_out @ WfR.T + bf     (Wf = [WfL | WfR])

g/beta are folded into the fusion weights on the host:
    out = z1 @ (WfL*g1).T + z2 @ (WfR*g2).T + (bf + beta1@WfL.T + beta2@WfR.T)
where z = (a - mean(a)) * rsqrt(var(a) + eps) is the bare standardization.

The host pre-packs both modalities into one tensor per layout so each
128-row block needs exactly two load DMAs:
  xbn : [rows, 2*D] bf16 = [x1 | x2] natural rows, pre-scaled by SCALE
        (LN is scale-invariant; eps is scaled by SCALE^2 to stay exact)
  xbt : [blk*128 + p, m*D + c*128 + r] fp8 = x_m[blk*128 + r, c*128 + p]
        for m in (x2, x1) - transposed per-block chunks, matmul order
Attention weights are fp8-e4m3 scaled by SCALE (escapes e4m3 subnormals).
Fusion weights are bf16 except z1 k-chunks 0,1, which run as one fp8
DoubleRow matmul per 512-half with both operands at natural scale (so
the f32 PSUM accumulation matches the bf16 chunks) and are emitted last
in the accumulation group so the zt->fp8 cast is off the critical path.
Rel err 0.0105 -> 0.0157 (gate 2e-2); 4 fp8 chunks would fail (0.022).

Per 128-row block:
  attention matmul in fp8 DoubleRow (x.T stationary [128,2,128], W.T
  moving [128,2,512], f32 PSUM) -> residual + LN stats on DVE ->
  z = a*rstd - mu*rstd on ScalarE into one [128, 2, D] tile -> single
  [128, 2048] transpose via the DMA xbar (SBUF->SBUF) -> bf16 fused
  matmul, emitted SKEW blocks late so the in-order PE stream never
  waits on the transpose chain -> f32 store.

Scheduling (the original strip kernel lost ~55us of PE time to DMA
cold-start, SP-sequencer congestion, and p-state re-ramps):
  - per-block loads in first-use order with a 4-block prefetch window,
    so the first attention matmul only waits on w1t + one 256KB chunk
  - loads + stores ride the SP HWDGE queue; the z-transpose rides the
    Activation HWDGE queue (dispatch costs ~600ns per DMA and in-order
    semaphore waits block everything behind them, so the transpose
    must not queue behind loads; its only wait is on z, produced by
    the same engine). Only the transpose goes there: load/store waits
    on the Activation queue couple the z critical path to tensor
    progress and lose ~100us.
  - a burst of dummy matmuls during the initial DMA wait ramps the PE
    clock (0.65 -> 2.4 GHz takes ~3us of continuous execution)
gpsimd (SWDGE) stays unused: its loads concurrent with DMA-transposes
wedge the device (unrecoverable).

Sharding: pure data parallel over the batch dim across 8 NeuronCores.
"""

import numpy as np
import ml_dtypes

import concourse.bass as bass
import concourse.mybir as mybir
import concourse.tile as tile
from concourse import bacc

B, D = 32768, 1024
N_CORES = 8
ROWS_PER_CORE = B // N_CORES
EPS = 1e-5
SCALE = 16.0  # fp8 scaling: w,x_nat carry x16; LN is scale-invariant (eps scaled too)
F32 = mybir.dt.float32
BF16 = mybir.dt.bfloat16
FP8 = mybir.dt.float8e4
BLK = 128  # row block (psum partition tile)
AF = mybir.ActivationFunctionType
ALU = mybir.AluOpType
DR = mybir.MatmulPerfMode.DoubleRow


def build_program(n_rows=ROWS_PER_CORE, use_b1=False, use_b2=False, use_bf=False):
    nc = bacc.Bacc("TRN2", target_bir_lowering=False, debug=False)
    n_blk = n_rows // BLK
    xbn = nc.dram_tensor("xbn", (n_rows, 2 * D), BF16, kind="ExternalInput").ap()
    xbt = nc.dram_tensor("xbt", (n_blk * 128, 2 * D), FP8, kind="ExternalInput").ap()
    w1t = nc.dram_tensor("w1t", (128, 8 * D), FP8, kind="ExternalInput").ap()
    w2t = nc.dram_tensor("w2t", (128, 8 * D), FP8, kind="ExternalInput").ap()
    wflt = nc.dram_tensor("wflt", (128, 8 * D), BF16, kind="ExternalInput").ap()
    wf8 = nc.dram_tensor("wf8", (128, 2 * D), FP8, kind="ExternalInput").ap()
    wfrt = nc.dram_tensor("wfrt", (128, 8 * D), BF16, kind="ExternalInput").ap()
    b1 = nc.dram_tensor("b1", (D,), F32, kind="ExternalInput").ap() if use_b1 else None
    b2 = nc.dram_tensor("b2", (D,), F32, kind="ExternalInput").ap() if use_b2 else None
    bfp = (
        nc.dram_tensor("bfp", (D,), F32, kind="ExternalInput").ap() if use_bf else None
    )
    out = nc.dram_tensor("out", (n_rows, D), F32, kind="ExternalOutput").ap()

    with tile.TileContext(nc) as tc:
        from contextlib import ExitStack

        with ExitStack() as ctx:
            consts = ctx.enter_context(tc.tile_pool(name="consts", bufs=1))
            xn_pool = ctx.enter_context(tc.tile_pool(name="xn", bufs=9))
            xt_pool = ctx.enter_context(tc.tile_pool(name="xt", bufs=9))
            work = ctx.enter_context(tc.tile_pool(name="work", bufs=6))
            zpool = ctx.enter_context(tc.tile_pool(name="z", bufs=6))
            ztpool = ctx.enter_context(tc.tile_pool(name="zt", bufs=10))
            opool = ctx.enter_context(tc.tile_pool(name="o", bufs=3))
            stats = ctx.enter_context(tc.tile_pool(name="stats", bufs=8))
            psum_mm = ctx.enter_context(
                tc.tile_pool(name="psum_mm", bufs=2, space="PSUM")
            )
            psum_o = ctx.enter_context(
                tc.tile_pool(name="psum_o", bufs=2, space="PSUM")
            )

            # --- PE clock warmup: dummy matmuls during the initial DMA wait.
            # The PE runs at 0.65/1.2 GHz until ~3us of continuous execution.
            warm = consts.tile([128, 256], BF16)
            nc.vector.memset(warm, 0.0)
            wps = psum_mm.tile([128, D], F32, name="ps_attn")
            for i in range(18):
                nc.tensor.matmul(
                    wps[:, 0:256], warm[:, 0:128], warm, start=True, stop=True
                )

            loads = {}  # (name, blk) -> tile

            def load_xt(j, eng=None):
                # transposed fp8 chunks [din_chunk=128, m, c, r=128]: m=0 is
                # x2 (attended by branch 0), m=1 is x1
                t = xt_pool.tile([128, 2, 8, BLK], FP8, name="xt")
                (eng or nc.sync).dma_start(
                    t,
                    xbt[j * 128 : (j + 1) * 128, :].rearrange(
                        "p (m c r) -> p m c r", m=2, c=8
                    ),
                )
                loads[("xt", j)] = t

            def load_xn(j, eng=None):
                # natural bf16 rows for the residual adds: m=0 is x1, m=1 is x2
                t = xn_pool.tile([128, 2, D], BF16, name="xn")
                (eng or nc.sync).dma_start(
                    t,
                    xbn[j * BLK : (j + 1) * BLK, :].rearrange("p (m n) -> p m n", m=2),
                )
                loads[("xn", j)] = t

            def load_block(j):
                load_xt(j)
                load_xn(j)

            # --- constants / weights. Each DMA dispatch serializes ~0.6-2.6us
            # on its sequencer and dispatch only starts at ~7us, so the cold
            # start is dispatch-bound: split the pre-staged loads across BOTH
            # HWDGE sequencers (they dispatch in parallel; none of these have
            # waits) in first-use order. The 4MB of fusion weights (first
            # needed by F(0) at ~30us) go last.
            def load_w(name_src, dt, eng, wname):
                t = consts.tile([128, 8, D], dt, name=wname)
                eng.dma_start(t, name_src.rearrange("p (c n) -> p c n", c=8))
                return t

            load_xt(0)
            w1t_sb = load_w(w1t, FP8, nc.sync, "w1t")
            load_xn(0)
            load_xt(1)
            w2t_sb = load_w(w2t, FP8, nc.sync, "w2t")
            load_xn(1)
            # fusion weights EARLY: F(0) must be runnable by ~27us so the
            # fused matmuls interleave with attention from the start -- an
            # attention-only burst outruns the z-chain (~6.3us/block) and
            # the resulting transpose backlog stalls the PE until ~130us
            wflt_sb = load_w(wflt, BF16, nc.sync, "wflt")
            wfrt_sb = load_w(wfrt, BF16, nc.sync, "wfrt")
            wf8_sb = consts.tile([128, 2, D], FP8, name="wf8")
            nc.sync.dma_start(wf8_sb, wf8.rearrange("p (c n) -> p c n", c=2))
            for _j in range(2, 7):
                load_block(_j)
            eps_sb = consts.tile([128, 1], F32)
            nc.vector.memset(eps_sb, EPS * SCALE * SCALE)
            b1_sb = b2_sb = bf_sb = None
            if use_b1:
                b1_sb = consts.tile([128, D], F32)
                nc.sync.dma_start(b1_sb, b1.partition_broadcast(128))
            if use_b2:
                b2_sb = consts.tile([128, D], F32)
                nc.sync.dma_start(b2_sb, b2.partition_broadcast(128))
            if use_bf:
                bf_sb = consts.tile([128, D], F32)
                nc.sync.dma_start(bf_sb, bfp.partition_broadcast(128))

            PF = 7  # blocks of load prefetch ahead of compute
            SKEW = 5  # blocks the fused matmul lags behind attention
            pending = []

            def emit_fused(r, zt, zt8):
                # out = z1 @ WfL'.T + z2 @ WfR'.T (+ bf')
                # z1 k-chunks 0,1 run as ONE fp8 DoubleRow matmul per half
                # (operands at natural scale so the f32 PSUM accumulation
                # matches the bf16 chunks; rel err 0.0105 -> 0.0152, gate 2e-2)
                po = psum_o.tile([128, D], F32, name="po")
                for br in range(2):
                    wt = wflt_sb if br == 0 else wfrt_sb
                    for c in range(2 if br == 0 else 0, 8):
                        lhsT = zt[:, 8 * br + c, :]
                        nc.tensor.matmul(
                            po[:, 0:512],
                            lhsT,
                            wt[:, c, 0:512],
                            start=(br == 0 and c == 2),
                            stop=False,
                        )
                        nc.tensor.matmul(
                            po[:, 512:1024],
                            lhsT,
                            wt[:, c, 512:1024],
                            start=(br == 0 and c == 2),
                            stop=False,
                        )
                nc.tensor.matmul(
                    po[:, 0:512], zt8, wf8_sb[:, :, 0:512],
                    start=False, stop=True, perf_mode=DR,
                )
                nc.tensor.matmul(
                    po[:, 512:1024], zt8, wf8_sb[:, :, 512:1024],
                    start=False, stop=True, perf_mode=DR,
                )
                o = opool.tile([128, D], F32, name="o")
                if bf_sb is not None:
                    nc.vector.tensor_add(o, po, bf_sb)
                else:
                    # on DVE, not ScalarE: a copy on the scalar engine makes
                    # the next block's z wait on fusion matmul completion
                    # (in-order engine stream), closing a feedback cycle with
                    # period > the 10.85us block time
                    nc.vector.tensor_copy(o, po)
                nc.sync.dma_start(out[r : r + BLK, :], o)

            for j in range(n_blk):
                if len(pending) > SKEW:
                    emit_fused(*pending.pop(0))
                if j + PF < n_blk:
                    load_block(j + PF)
                r = j * BLK
                z = zpool.tile([128, 2, D], BF16, name="z")
                for br in range(2):
                    xt_op = loads[("xt", j)][:, br]  # attended modality
                    res = loads[("xn", j)][:, br]
                    wt = w1t_sb if br == 0 else w2t_sb
                    bias_sb = b1_sb if br == 0 else b2_sb
                    # attended = x_other @ W.T    [128 rows, 1024]
                    # fp8 DoubleRow: 2 k-chunks per matmul (virtual K=256)
                    ps = psum_mm.tile([128, D], F32, name="ps_attn")
                    for c in range(4):
                        lhsT = xt_op[:, 2 * c : 2 * c + 2, :]
                        nc.tensor.matmul(
                            ps[:, 0:512],
                            lhsT,
                            wt[:, 2 * c : 2 * c + 2, 0:512],
                            start=(c == 0),
                            stop=(c == 3),
                            perf_mode=DR,
                        )
                        nc.tensor.matmul(
                            ps[:, 512:1024],
                            lhsT,
                            wt[:, 2 * c : 2 * c + 2, 512:1024],
                            start=(c == 0),
                            stop=(c == 3),
                            perf_mode=DR,
                        )
                    # a = residual + attended (+ bias), bf16 for 2x DVE reads
                    a = work.tile([128, D], BF16, name="a")
                    nc.vector.tensor_add(a, ps, res)
                    if bias_sb is not None:
                        nc.vector.tensor_add(a, a, bias_sb)
                    # layernorm statistics
                    st = stats.tile([128, 2, 6], F32, name="st")
                    nc.vector.bn_stats(st[:, 0, :], a[:, 0:512])
                    nc.vector.bn_stats(st[:, 1, :], a[:, 512:1024])
                    mv = stats.tile([128, 2], F32, name="mv")
                    nc.vector.bn_aggr(mv, st)
                    rstd = stats.tile([128, 1], F32, name="rstd")
                    nc.scalar.activation(rstd, mv[:, 1:2], AF.Sqrt, bias=eps_sb)
                    nc.vector.reciprocal(rstd, rstd)
                    # z = a*rstd - mean*rstd on the scalar engine
                    nmr = stats.tile([128, 1], F32, name="nmr")
                    nc.vector.tensor_scalar(
                        nmr, mv[:, 0:1], rstd, -1.0, op0=ALU.mult, op1=ALU.mult
                    )
                    nc.vector.tensor_scalar(
                        z[:, br], a, rstd, nmr, op0=ALU.mult, op1=ALU.add
                    )
                # transpose both halves of z in one DMA xbar pass
                # (SBUF->SBUF, 2-byte): zt[p, 8*br + c, r] = z[r, br, c*128 + p]
                zt = ztpool.tile([128, 16, BLK], BF16, name="zt")
                nc.scalar.dma_start(zt, z.rearrange("p m n -> p (m n)"), transpose=True)
                zt8 = ztpool.tile([128, 2, BLK], FP8, name="zt8")
                nc.scalar.copy(zt8, zt[:, 0:2, :])
                pending.append((r, zt, zt8))
            for args in pending:
                emit_fused(*args)
    nc.compile()
    return nc


def _host_prep(Wv1, bv1, Wo1, bo1, Wv2, bv2, Wo2, bo2, g1, beta1, g2, beta2, Wf, bf):
    f32 = np.float32
    bfd = ml_dtypes.bfloat16
    Wv1, Wo1, Wv2, Wo2, Wf = (np.asarray(a, f32) for a in (Wv1, Wo1, Wv2, Wo2, Wf))
    bv1, bo1, bv2, bo2, bf = (np.asarray(a, f32) for a in (bv1, bo1, bv2, bo2, bf))
    g1, beta1, g2, beta2 = (np.asarray(a, f32) for a in (g1, beta1, g2, beta2))

    W1 = Wo1 @ Wv1  # [dout, din]
    W2 = Wo2 @ Wv2
    b1 = bv1 @ Wo1.T + bo1
    b2 = bv2 @ Wo2.T + bo2
    WfL = Wf[:, :D] * g1[None, :]
    WfR = Wf[:, D:] * g2[None, :]
    bfp = bf + beta1 @ Wf[:, :D].T + beta2 @ Wf[:, D:].T

    f8 = ml_dtypes.float8_e4m3

    def wlay(wT, dt):
        # SBUF layout [p, c, n] flattened to [128, 8*D]: row p holds chunk
        # rows p, 128+p, ... so the DMA is one contiguous 8KB line per row
        return np.ascontiguousarray(
            wT.reshape(8, 128, D).transpose(1, 0, 2).reshape(128, 8 * D)
        ).astype(dt)

    weights = {
        "w1t": wlay(SCALE * W1.T, f8),
        "w2t": wlay(SCALE * W2.T, f8),
        "wflt": wlay(WfL.T, bfd),
        "wfrt": wlay(WfR.T, bfd),
        # z1 k-chunks 0,1 of the fused matmul in fp8 at natural scale
        "wf8": np.ascontiguousarray(
            WfL.T[:256].reshape(2, 128, D).transpose(1, 0, 2).reshape(128, 2 * D)
        ).astype(f8),
    }
    use_b1 = bool(np.any(b1 != 0))
    use_b2 = bool(np.any(b2 != 0))
    use_bf = bool(np.any(bfp != 0))
    if use_b1:
        weights["b1"] = SCALE * b1
    if use_b2:
        weights["b2"] = SCALE * b2
    if use_bf:
        weights["bfp"] = bfp
    return weights, use_b1, use_b2, use_bf


def _xt_blocks(x_core):
    """Per-block transposed fp8 chunks: [blk*128 + p, c*128 + r] =
    x[blk*128 + r, c*128 + p], flattened to [n_blk*128, D]."""
    n_blk = x_core.shape[0] // BLK
    return np.ascontiguousarray(
        x_core.astype(ml_dtypes.float8_e4m3)
        .reshape(n_blk, BLK, 8, 128)
        .transpose(0, 3, 2, 1)
    ).reshape(n_blk * 128, D)


def kernel(
    eeg_emb,
    ecg_emb,
    Wv1,
    bv1,
    Wo1,
    bo1,
    Wv2,
    bv2,
    Wo2,
    bo2,
    g1,
    beta1,
    g2,
    beta2,
    Wf,
    bf,
    _run_kwargs=None,
):
    from concourse.bass_utils import run_bass_kernel_spmd

    eeg = np.asarray(eeg_emb, np.float32)
    ecg = np.asarray(ecg_emb, np.float32)
    weights, use_b1, use_b2, use_bf = _host_prep(
        Wv1, bv1, Wo1, bo1, Wv2, bv2, Wo2, bo2, g1, beta1, g2, beta2, Wf, bf
    )
    nc = build_program(ROWS_PER_CORE, use_b1, use_b2, use_bf)
    bfd = ml_dtypes.bfloat16
    in_maps = []
    for i in range(N_CORES):
        sl = slice(i * ROWS_PER_CORE, (i + 1) * ROWS_PER_CORE)
        e, c = eeg[sl], ecg[sl]
        xbn = np.concatenate(
            [(SCALE * e).astype(bfd), (SCALE * c).astype(bfd)], axis=1
        )
        xbt = np.concatenate([_xt_blocks(c), _xt_blocks(e)], axis=1)
        in_maps.append({"xbn": xbn, "xbt": xbt, **weights})
    res = run_bass_kernel_spmd(
        nc, in_maps, core_ids=list(range(N_CORES)), **(_run_kwargs or {})
    )
    out = np.concatenate([r["out"] for r in res.results], axis=0)
    if _run_kwargs:
        kernel.last_results = res
    return out
